# revision 2
# baseline (speedup 1.0000x reference)
"""Trainium2 Bass kernel for the integrate-and-fire "Integrator" layer.

Semantics (matches the JAX reference exactly):
  input  x  [4, 200, 64, 64, 8] f32, split into 2 independent time chunks of 100.
  Per neuron and chunk: W_t = V_{t-1} + x_t; spike iff W_t > 2; V_t = 0 on
  spike else W_t.  Output: spike raster, permuted to [B, T, W, C, H] f32.

v4: one custom DVE instruction per time step.  The state carried between
steps is the PRE-reset membrane W, so the whole update fuses into

    W_t = select(W_{t-1} <= theta, W_{t-1}, 0) + x_t

one [128, 256] fp32 op (~425 ns) per step, written straight into the DMA
staging tile.  The ScalarEngine converts each group of W slots to spike
masks in one big activation op, sign(W - 2) -> fp8e4 (exact: W in [1,4]
makes W-2 exact fp32 by Sterbenz; sign of a correctly-rounded subtract is
the sign of the true difference; +/-1, 0 are exact in fp8).  The host maps
sign > 0 -> spike.  Input DMAs ride the Sync HWDGE ring while output DMAs
ride the Scalar HWDGE ring, so input prefetch never queues behind stores.
262144 independent sequences, 32768 per core as [128 part, 256 free].
"""

import numpy as np

from concourse import bacc, bass, mybir
from concourse import dve_ops as _dve_ops
from concourse.dve_ops import DveOp, OPS
from concourse.dve_spec import Spec, Src0, Src1, C0, Zero, select, lower
from concourse.dve_uop import DveOpSpec
from concourse.tile import TileContext
from concourse.bass_utils import run_bass_kernel_spmd

_THETA = 2.0
_T = 100   # chunk length (time steps per independent sequence)
_P = 128   # SBUF partitions
_F = 256   # sequences per partition per core
_NC = 8
_GS = [2, 4, 6, 8] + [10] * 7 + [6, 3, 1]   # DMA group sizes, sum = _T

_B, _TT, _H, _W, _C = 4, 200, 64, 64, 8

_IAF_NAME = "IAF_STEP_ANT"


def _register_iaf_op():
    """Register the fused integrate-and-fire step as a custom DVE op:
    out = select(in0 <= s0, in0, 0) + in1.  Idempotent."""
    for op in OPS:
        if op.name == _IAF_NAME:
            return op
    spec = Spec(
        body=select(Src0 <= C0, Src0, Zero) + Src1,
        reference=lambda in0, in1, s0: np.where(in0 <= s0, in0, 0.0) + in1,
    )
    row = _dve_ops._CUSTOM_DVE_ROW_BASE + len(OPS)
    _dve_ops._SUB_OPCODE_FOR_NAME[_IAF_NAME] = row
    shas = {}
    for ver in ("v3", "v4"):
        shas[ver] = DveOpSpec(
            name=_IAF_NAME, opcode=row, uops=lower(spec, ver=ver), rd1_en=True
        ).sha(ver)
    op = DveOp(_IAF_NAME, spec, subdim=False, uops_sha=shas)
    OPS.append(op)
    return op


_IAF_OP = _register_iaf_op()


def _build():
    nc = bacc.Bacc("TRN2", target_bir_lowering=False, debug=False)
    x = nc.declare_dram_parameter("x", [_P, _T, _F], mybir.dt.float32, isOutput=False)
    s = nc.declare_dram_parameter("s", [_P, _T, _F], mybir.dt.float8e4, isOutput=True)
    with TileContext(nc) as tc:
        with (
            tc.tile_pool(name="xin", bufs=len(_GS)) as xpool,
            tc.tile_pool(name="wstage", bufs=3) as wpool,
            tc.tile_pool(name="sout", bufs=3) as spool,
            tc.tile_pool(name="state", bufs=1) as stpool,
        ):
            Z = stpool.tile([_P, _F], mybir.dt.float32, tag="Z")
            nc.vector.memset(Z[:], 0.0)
            bias = stpool.tile([_P, 1], mybir.dt.float32, tag="bias")
            nc.vector.memset(bias[:], -_THETA)
            prevW = Z[:]
            t0 = 0
            for gs in _GS:
                xt = xpool.tile([_P, gs, _F], mybir.dt.float32, tag="x")
                nc.sync.dma_start(out=xt[:], in_=x[:, t0:t0 + gs, :])
                wt = wpool.tile([_P, gs, _F], mybir.dt.float32, tag="w")
                st = spool.tile([_P, gs, _F], mybir.dt.float8e4, tag="s")
                for k in range(gs):
                    # W_t = select(W_{t-1} <= theta, W_{t-1}, 0) + x_t
                    nc.vector._custom_dve(
                        _IAF_OP,
                        out=wt[:, k, :],
                        in0=prevW,
                        in1=xt[:, k, :],
                        s0=_THETA,
                    )
                    prevW = wt[:, k, :]
                # one big mask op per group: sign(W - theta) -> bf16
                nc.scalar.activation(
                    out=st[:], in_=wt[:],
                    func=mybir.ActivationFunctionType.Sign, bias=bias[:])
                nc.scalar.dma_start(out=s[:, t0:t0 + gs, :], in_=st[:])
                t0 += gs
    return nc


def _shard(x):
    # [B, 200, H, W, C] -> per-core [128, 100, 256] f32, sequence-major
    xr = (
        x.reshape(_B, 2, _T, _H, _W, _C)
        .transpose(0, 1, 3, 4, 5, 2)  # [b, chunk, h, w, c, t]
        .reshape(-1, _T)              # [262144, 100]
    )
    per_core = xr.reshape(_NC, _P, _F, _T).transpose(0, 1, 3, 2)  # [8,128,100,256]
    return [np.ascontiguousarray(per_core[c]) for c in range(_NC)]


def _unshard(core_outs):
    # list of [128, 100, 256] fp8 sign(W-2) -> spike raster [B, T, W, C, H] f32
    sg = np.stack([np.asarray(o, dtype=np.float32) for o in core_outs])
    sp = (sg > 0).astype(np.float32)
    sp = sp.transpose(0, 1, 3, 2).reshape(_B, 2, _H, _W, _C, _T)  # [b,k,h,w,c,t]
    out = sp.transpose(0, 1, 5, 3, 4, 2).reshape(_B, _TT, _W, _C, _H)
    return np.ascontiguousarray(out)


def _run(x, trace=False):
    nc = _build()
    nc.finalize()
    in_maps = [{"x": xc} for xc in _shard(np.asarray(x, dtype=np.float32))]
    res = run_bass_kernel_spmd(nc, in_maps, core_ids=list(range(_NC)), trace=trace)
    out = _unshard([r["s"] for r in res.results])
    return out, res


def kernel(inputs):
    out, _ = _run(inputs, trace=False)
    return out
